# revision 24
# baseline (speedup 1.0000x reference)
"""Trainium2 Bass kernel for nn_BoundaryGreenBranch.

Math (reference):
    bf = relu(relu(bi @ W0 + b0) @ W1 + b1)            # (B, NBC, HID) tiny
    a  = bf @ G0w[:HID] + G0b                          # (B, NBC, HID) tiny
    c  = coords @ G0w[HID:]                            # (B, NINT, HID) small
    h1 = relu(a[:,:,None,:] + c[:,None,:,:])           # (B, NBC, NINT, HID) huge
    h2 = relu(h1 @ G1w + G1b)                          # huge
    u  = (h2 @ G2w + G2b).sum(bc) / NBC                # (B, NINT, 1)

Sharding: 8 cores = 4 batches x 2 halves of NBC (64 bc each). Host does the
tiny encoder stages; each core does its 64bc x 4096int x 64hid block fully
on-chip; host sums the two partial u's per batch (the bc all-reduce).

On-core layout (per quad of 4 bc, pairs packed 2-up on 128 partitions):
    pass1: h1 tiles [128, 4096] fp16, relu(cT_dup + a'_pair) with a' as a
        per-partition scalar: DVE tensor_scalar (4x mode) for most tiles,
        ACT for the prologue h1b (fills ACT's idle startup window), GPSIMD
        for a few h1b tiles (emitted 2 quads ahead to cover its latency).
    G1: 4 concurrent quadrant matmuls (tile_position) since K=M=64 fills the
        128x128 PE array -> h2pre in PSUM [128, 1024] (2 banks, 3 slots).
    pass2: relu(h2pre + G1b) PSUM->SBUF fp16, split chunk-by-chunk between
        ACT (activation bias trick, ~1147ns) and DVE (dual-op tensor_scalar,
        ~1270ns) by a greedy virtual-clock balance that also accounts for
        pass1 -- so DVE picks up extra chunks in the last quads when pass1
        runs dry instead of idling.
    G2: lhsT=[G2w;G2w] [128,1] matmuls accumulate the sum over bc in PSUM u
        slots (8 chunks -> 2 banks x 4 col-group partitions); emitted in
        4-column-group concurrent batches, lagged a quad so the PE never
        waits on a late pass2. fp16 everywhere on the 16-bit path: same
        speed as bf16 but ~8x lower error (~4e-4 rel).

Small constants ride in two DMAs (f32 a-pairs+G1b, f16 G1w+G2w); cT goes as
8 x 512-col pieces split over the sync+gpsimd issue queues. Output u leaves
as one strided-partition DMA per PSUM accumulator bank.
"""

import numpy as np

B, NBC, HID = 4, 128, 64
NINT = 4096
NCORES = 8
NQUAD = 16  # quads of 4 bc per core (64 bc / 4)
NCH = 8  # interior chunks of 512
CHW = 512  # chunk width

# GPSIMD's stock tensor_scalar ucode measures ~17 cyc/elem (58us per h1
# tile) -- useless for pass1, so no quads are assigned to it.
GP_QUADS = frozenset()

_PROG = {}


def _build_program():
    import concourse.bacc as bacc
    import concourse.tile as tile
    from concourse import mybir

    f32 = mybir.dt.float32
    f16 = mybir.dt.float16
    Relu = mybir.ActivationFunctionType.Relu
    add = mybir.AluOpType.add
    mx = mybir.AluOpType.max

    nc = bacc.Bacc("TRN2")
    d_ct = nc.declare_dram_parameter("ctdup", [128, NINT], f16, isOutput=False)
    d_cstf = nc.declare_dram_parameter("cstf", [128, 33], f32, isOutput=False)
    d_csth = nc.declare_dram_parameter("csth", [128, 65], f16, isOutput=False)
    d_u = nc.declare_dram_parameter("upart", [NCH, CHW], f32, isOutput=True)

    with tile.TileContext(nc) as tc:
        with (
            tc.tile_pool(name="const", bufs=1) as const,
            tc.tile_pool(name="h1", bufs=4) as h1pool,
            tc.tile_pool(name="h2", bufs=20) as h2pool,
            tc.tile_pool(name="ps", bufs=3, space="PSUM") as pspool,
            tc.tile_pool(name="psu", bufs=1, space="PSUM") as upool,
            tc.tile_pool(name="outp", bufs=1) as outpool,
        ):
            # DMA order: the first 512 cT columns gate the whole pipeline,
            # so they go as two 256-col DMAs in parallel on sync+gpsimd;
            # the small consts ride the otherwise-idle scalar queue.
            cstf = const.tile([128, 33], f32)
            csth = const.tile([128, 65], f16)
            sb_ct = const.tile([128, NINT], f16)

            def ctp(i):
                return slice(i * 512, (i + 1) * 512)

            nc.sync.dma_start(out=sb_ct[:, 0:256], in_=d_ct[:, 0:256])
            nc.gpsimd.dma_start(out=sb_ct[:, 256:512], in_=d_ct[:, 256:512])
            nc.scalar.dma_start(out=cstf[:], in_=d_cstf[:])
            nc.scalar.dma_start(out=csth[:], in_=d_csth[:])
            for i in (1, 3, 5, 7):
                nc.sync.dma_start(out=sb_ct[:, ctp(i)], in_=d_ct[:, ctp(i)])
            for i in (2, 4, 6):
                nc.gpsimd.dma_start(out=sb_ct[:, ctp(i)], in_=d_ct[:, ctp(i)])

            sb_ap = cstf[:, 0:32]
            sb_g1b = cstf[:, 32:33]
            sb_g1w = csth[:, 0:64]
            sb_g2w = csth[:, 64:65]

            # warm the ACT Relu table while the cT DMA runs
            dummy = const.tile([128, 1], f32)
            nc.scalar.activation(out=dummy[:], in_=sb_g1b, func=Relu)

            psu = [
                upool.tile([128, CHW], f32, name=f"u{i}", tag=f"u{i}")
                for i in range(2)
            ]

            def emit_g2_batch(q, cbase, h2s4):
                """8 G2 matmuls for chunks cbase..cbase+3: per tile-half, the
                4 chunks target 4 distinct PE column groups -> concurrent.
                Keep these batched: G2's full-height column LDWEIGHTS
                conflicts with the G1 quadrants, so each batch is one PE
                serialization point (spreading them per-chunk measured 10%
                slower end-to-end)."""
                ub = psu[cbase // 4]
                for half in range(2):
                    sl = slice(half * CHW, (half + 1) * CHW)
                    for k in range(4):
                        j = 32 * k
                        nc.tensor.matmul(
                            ub[j : j + 1, :], sb_g2w, h2s4[k][:, sl],
                            start=(q == 0 and half == 0),
                            stop=(q == NQUAD - 1 and half == 1),
                            tile_position=(0, j),
                        )

            h1t = {}

            def h1(qq):
                if qq not in h1t:
                    h1t[qq] = (
                        h1pool.tile([128, NINT], f16, name="h1a", tag="h1a"),
                        h1pool.tile([128, NINT], f16, name="h1b", tag="h1b"),
                    )
                return h1t[qq]

            def pass1(eng, tile_, col):
                if eng == "V":
                    nc.vector.tensor_scalar(
                        out=tile_, in0=sb_ct[:, 0 : tile_.shape[1]],
                        scalar1=sb_ap[:, col : col + 1], scalar2=0.0,
                        op0=add, op1=mx,
                    )
                elif eng == "A":
                    nc.scalar.activation(
                        out=tile_, in_=sb_ct[:, 0 : tile_.shape[1]], func=Relu,
                        bias=sb_ap[:, col : col + 1], scale=1.0,
                    )
                else:
                    nc.gpsimd.tensor_scalar(
                        out=tile_, in0=sb_ct[:, 0 : tile_.shape[1]],
                        scalar1=sb_ap[:, col : col + 1], scalar2=0.0,
                        op0=add, op1=mx,
                    )

            # prologue quad 0 on DVE, sized to DMA piece arrival: 512-col
            # ops first so chunk-0 matmuls start as soon as cT piece 0 lands
            a0, b0 = h1(0)
            for lo, hi in ((0, 512), (512, 1024), (1024, 2048), (2048, 4096)):
                for tile_, col in ((a0, 0), (b0, 1)):
                    nc.vector.tensor_scalar(
                        out=tile_[:, lo:hi], in0=sb_ct[:, lo:hi],
                        scalar1=sb_ap[:, col : col + 1], scalar2=0.0,
                        op0=add, op1=mx,
                    )

            prev_h2s = None  # previous quad's h2 tiles, G2 lagged a quad
            for q in range(NQUAD):
                h1a, h1b = h1(q)
                # DVE's pass2 chunk share: 2.5 avg in steady state (balanced
                # against its pass1 load), more in the last quads once pass1
                # emission dries up and ACT would otherwise become the tail.
                if q == NQUAD - 1:
                    dve_set = (1, 3, 5, 7)
                elif q == NQUAD - 2:
                    dve_set = (2, 5, 7)
                else:
                    # LP balance point is 2.67 DVE chunks/quad against its
                    # pass1 load: (2,5) every third quad, (2,5,7) otherwise
                    dve_set = (2, 5) if q % 3 == 0 else (2, 5, 7)
                h2s = []
                for c in range(NCH):
                    sl = slice(c * CHW, (c + 1) * CHW)
                    ps = pspool.tile([128, 2 * CHW], f32, tag="h2pre")
                    nc.tensor.matmul(
                        ps[0:64, 0:CHW], sb_g1w[0:64, :], h1a[0:64, sl],
                        start=True, stop=True, tile_position=(0, 0),
                    )
                    nc.tensor.matmul(
                        ps[64:128, 0:CHW], sb_g1w[64:128, :], h1a[64:128, sl],
                        start=True, stop=True, tile_position=(64, 64),
                    )
                    nc.tensor.matmul(
                        ps[64:128, CHW : 2 * CHW], sb_g1w[0:64, :], h1b[0:64, sl],
                        start=True, stop=True, tile_position=(0, 64),
                    )
                    nc.tensor.matmul(
                        ps[0:64, CHW : 2 * CHW], sb_g1w[64:128, :], h1b[64:128, sl],
                        start=True, stop=True, tile_position=(64, 0),
                    )
                    h2 = h2pool.tile([128, 2 * CHW], f16, tag="h2")
                    if c in dve_set:
                        nc.vector.tensor_scalar(
                            out=h2[:], in0=ps[:],
                            scalar1=sb_g1b, scalar2=0.0, op0=add, op1=mx,
                        )
                    else:
                        nc.scalar.activation(
                            out=h2[:], in_=ps[:], func=Relu,
                            bias=sb_g1b, scale=1.0,
                        )
                    h2s.append(h2)
                    if c == 1 and q + 1 < NQUAD:
                        pass1("V", h1(q + 1)[0][:], 2 * (q + 1))
                    if c == 4 and q + 1 < NQUAD:
                        pass1("V", h1(q + 1)[1][:], 2 * (q + 1) + 1)
                    if c == 1 and prev_h2s is not None:
                        emit_g2_batch(q - 1, 4, prev_h2s[4:8])
                emit_g2_batch(q, 0, h2s[0:4])
                prev_h2s = h2s

            def evac_u(i):
                so = outpool.tile([128, CHW], f32, name=f"so{i}", tag=f"so{i}")
                if i == 0:
                    nc.vector.tensor_copy(out=so[:], in_=psu[i][:])
                else:
                    nc.scalar.copy(out=so[:], in_=psu[i][:])
                nc.sync.dma_start(
                    out=d_u[4 * i : 4 * i + 4, :], in_=so[::32, :]
                )

            # final U1 accumulation in two 4-MM column-group batches so the
            # (c4,c5) pair starts as soon as its pass2 lands instead of
            # waiting for c7's; U0 evac overlaps both.
            evac_u(0)
            for half in range(2):
                for k in (0, 1):
                    nc.tensor.matmul(
                        psu[1][32 * k : 32 * k + 1, :], sb_g2w,
                        prev_h2s[4 + k][:, half * CHW : (half + 1) * CHW],
                        start=False, stop=(half == 1),
                        tile_position=(0, 32 * k),
                    )
            for half in range(2):
                for k in (2, 3):
                    nc.tensor.matmul(
                        psu[1][32 * k : 32 * k + 1, :], sb_g2w,
                        prev_h2s[4 + k][:, half * CHW : (half + 1) * CHW],
                        start=False, stop=(half == 1),
                        tile_position=(0, 32 * k),
                    )
            evac_u(1)

    nc.compile()
    return nc


def _relu(x):
    return np.maximum(x, 0.0)


def _prepare_in_maps(
    boundary_info, interior_coords, W0, b0, W1, b1,
    G0w, G0b, G1w, G1b, G2w, G2b,
):
    f16 = np.float16
    bi = np.asarray(boundary_info, np.float32)
    coords = np.asarray(interior_coords, np.float32)
    W0, b0, W1, b1 = (np.asarray(t, np.float32) for t in (W0, b0, W1, b1))
    G0w, G0b, G1w, G1b, G2w, G2b = (
        np.asarray(t, np.float32) for t in (G0w, G0b, G1w, G1b, G2w, G2b)
    )

    # tiny encoder stages on host
    bf = _relu(bi @ W0 + b0)
    bf = _relu(bf @ W1 + b1)
    a = bf @ G0w[:HID] + G0b  # (B, NBC, HID)
    cint = coords @ G0w[HID:]  # (B, NINT, HID)

    g1w_sb = np.vstack([G1w, G1w]).astype(f16)
    g2w_sb = np.vstack([G2w, G2w]).astype(f16)
    g1b2 = np.concatenate([G1b, G1b]).reshape(128, 1).astype(np.float32)
    csth = np.ascontiguousarray(np.concatenate([g1w_sb, g2w_sb], axis=1))

    in_maps = []
    for core in range(NCORES):
        b, half = divmod(core, 2)
        cT = np.ascontiguousarray(cint[b].T)  # (64, 4096)
        ctdup = np.vstack([cT, cT]).astype(f16)
        asl = a[b, half * 64 : (half + 1) * 64]  # (64 bc, 64 hid)
        apairs = np.ascontiguousarray(asl.reshape(32, 128).T).astype(np.float32)
        cstf = np.ascontiguousarray(np.concatenate([apairs, g1b2], axis=1))
        in_maps.append({"ctdup": ctdup, "cstf": cstf, "csth": csth})
    return in_maps


def _run(in_maps, **kwargs):
    from concourse.bass_utils import run_bass_kernel_spmd

    if "nc" not in _PROG:
        _PROG["nc"] = _build_program()
    return run_bass_kernel_spmd(_PROG["nc"], in_maps, list(range(NCORES)), **kwargs)


def kernel(
    boundary_info, interior_coords, W0, b0, W1, b1,
    G0w, G0b, G1w, G1b, G2w, G2b, interior_h, interior_w,
):
    in_maps = _prepare_in_maps(
        boundary_info, interior_coords, W0, b0, W1, b1,
        G0w, G0b, G1w, G1b, G2w, G2b,
    )
    res = _run(in_maps)

    u = np.zeros((B, NINT), np.float64)
    for core in range(NCORES):
        b = core // 2
        u[b] += res.results[core]["upart"].reshape(NINT).astype(np.float64)
    u = (u / NBC + np.asarray(G2b, np.float32)[0]).astype(np.float32)
    return u.reshape(B, 1, int(interior_h), int(interior_w))


# revision 25
# speedup vs baseline: 1.0442x; 1.0442x over previous
"""Trainium2 Bass kernel for nn_BoundaryGreenBranch.

Math (reference):
    bf = relu(relu(bi @ W0 + b0) @ W1 + b1)            # (B, NBC, HID) tiny
    a  = bf @ G0w[:HID] + G0b                          # (B, NBC, HID) tiny
    c  = coords @ G0w[HID:]                            # (B, NINT, HID) small
    h1 = relu(a[:,:,None,:] + c[:,None,:,:])           # (B, NBC, NINT, HID) huge
    h2 = relu(h1 @ G1w + G1b)                          # huge
    u  = (h2 @ G2w + G2b).sum(bc) / NBC                # (B, NINT, 1)

Sharding: 8 cores = 4 batches x 2 halves of NBC (64 bc each). Host does the
tiny encoder stages; each core does its 64bc x 4096int x 64hid block fully
on-chip; host sums the two partial u's per batch (the bc all-reduce).

On-core layout (per quad of 4 bc, pairs packed 2-up on 128 partitions):
    pass1: h1 tiles [128, 4096] fp16, relu(cT_dup + a'_pair) with a' as a
        per-partition scalar: DVE tensor_scalar (4x mode) for most tiles,
        ACT for the prologue h1b (fills ACT's idle startup window), GPSIMD
        for a few h1b tiles (emitted 2 quads ahead to cover its latency).
    G1: 4 concurrent quadrant matmuls (tile_position) since K=M=64 fills the
        128x128 PE array -> h2pre in PSUM [128, 1024] (2 banks, 3 slots).
    pass2: relu(h2pre + G1b) PSUM->SBUF fp16, split chunk-by-chunk between
        ACT (activation bias trick, ~1147ns) and DVE (dual-op tensor_scalar,
        ~1270ns) by a greedy virtual-clock balance that also accounts for
        pass1 -- so DVE picks up extra chunks in the last quads when pass1
        runs dry instead of idling.
    G2: lhsT=[G2w;G2w] [128,1] matmuls accumulate the sum over bc in PSUM u
        slots (8 chunks -> 2 banks x 4 col-group partitions); emitted in
        4-column-group concurrent batches, lagged a quad so the PE never
        waits on a late pass2. fp16 everywhere on the 16-bit path: same
        speed as bf16 but ~8x lower error (~4e-4 rel).

Small constants ride in two DMAs (f32 a-pairs+G1b, f16 G1w+G2w); cT goes as
8 x 512-col pieces split over the sync+gpsimd issue queues. Output u leaves
as one strided-partition DMA per PSUM accumulator bank.
"""

import numpy as np

B, NBC, HID = 4, 128, 64
NINT = 4096
NCORES = 8
NQUAD = 16  # quads of 4 bc per core (64 bc / 4)
NCH = 8  # interior chunks of 512
CHW = 512  # chunk width

# GPSIMD's stock tensor_scalar ucode measures ~17 cyc/elem (58us per h1
# tile) -- useless for pass1, so no quads are assigned to it.
GP_QUADS = frozenset()

_PROG = {}


def _build_program():
    import concourse.bacc as bacc
    import concourse.tile as tile
    from concourse import mybir

    f32 = mybir.dt.float32
    f16 = mybir.dt.float16
    Relu = mybir.ActivationFunctionType.Relu
    add = mybir.AluOpType.add
    mx = mybir.AluOpType.max

    nc = bacc.Bacc("TRN2")
    d_ct = nc.declare_dram_parameter("ctdup", [128, NINT], f16, isOutput=False)
    d_cstf = nc.declare_dram_parameter("cstf", [128, 33], f32, isOutput=False)
    d_csth = nc.declare_dram_parameter("csth", [128, 65], f16, isOutput=False)
    d_u = nc.declare_dram_parameter("upart", [NCH, CHW], f32, isOutput=True)

    with tile.TileContext(nc) as tc:
        with (
            tc.tile_pool(name="const", bufs=1) as const,
            tc.tile_pool(name="h1", bufs=4) as h1pool,
            tc.tile_pool(name="h2", bufs=20) as h2pool,
            tc.tile_pool(name="ps", bufs=3, space="PSUM") as pspool,
            tc.tile_pool(name="psu", bufs=1, space="PSUM") as upool,
            tc.tile_pool(name="outp", bufs=1) as outpool,
        ):
            # DMA order: the first 512 cT columns gate the whole pipeline,
            # so they go as two 256-col DMAs in parallel on sync+gpsimd;
            # the small consts ride the otherwise-idle scalar queue.
            cstf = const.tile([128, 33], f32)
            csth = const.tile([128, 65], f16)
            sb_ct = const.tile([128, NINT], f16)

            def ctp(i):
                return slice(i * 512, (i + 1) * 512)

            nc.sync.dma_start(out=sb_ct[:, 0:256], in_=d_ct[:, 0:256])
            nc.gpsimd.dma_start(out=sb_ct[:, 256:512], in_=d_ct[:, 256:512])
            nc.scalar.dma_start(out=cstf[:], in_=d_cstf[:])
            nc.scalar.dma_start(out=csth[:], in_=d_csth[:])
            for i in (1, 3, 5, 7):
                nc.sync.dma_start(out=sb_ct[:, ctp(i)], in_=d_ct[:, ctp(i)])
            for i in (2, 4, 6):
                nc.gpsimd.dma_start(out=sb_ct[:, ctp(i)], in_=d_ct[:, ctp(i)])

            sb_ap = cstf[:, 0:32]
            sb_g1b = cstf[:, 32:33]
            sb_g1w = csth[:, 0:64]
            sb_g2w = csth[:, 64:65]

            # warm the ACT Relu table while the cT DMA runs
            dummy = const.tile([128, 1], f32)
            nc.scalar.activation(out=dummy[:], in_=sb_g1b, func=Relu)

            psu = [
                upool.tile([128, CHW], f32, name=f"u{i}", tag=f"u{i}")
                for i in range(2)
            ]

            def emit_g2_batch(q, cbase, h2s4):
                """8 G2 matmuls for chunks cbase..cbase+3: per tile-half, the
                4 chunks target 4 distinct PE column groups -> concurrent.
                Keep these batched: G2's full-height column LDWEIGHTS
                conflicts with the G1 quadrants, so each batch is one PE
                serialization point (spreading them per-chunk measured 10%
                slower end-to-end)."""
                ub = psu[cbase // 4]
                for half in range(2):
                    sl = slice(half * CHW, (half + 1) * CHW)
                    for k in range(4):
                        j = 32 * k
                        nc.tensor.matmul(
                            ub[j : j + 1, :], sb_g2w, h2s4[k][:, sl],
                            start=(q == 0 and half == 0),
                            stop=(q == NQUAD - 1 and half == 1),
                            tile_position=(0, j),
                        )

            h1t = {}

            def h1(qq):
                if qq not in h1t:
                    h1t[qq] = (
                        h1pool.tile([128, NINT], f16, name="h1a", tag="h1a"),
                        h1pool.tile([128, NINT], f16, name="h1b", tag="h1b"),
                    )
                return h1t[qq]

            def pass1(eng, tile_, col):
                if eng == "V":
                    nc.vector.tensor_scalar(
                        out=tile_, in0=sb_ct[:, 0 : tile_.shape[1]],
                        scalar1=sb_ap[:, col : col + 1], scalar2=0.0,
                        op0=add, op1=mx,
                    )
                elif eng == "A":
                    nc.scalar.activation(
                        out=tile_, in_=sb_ct[:, 0 : tile_.shape[1]], func=Relu,
                        bias=sb_ap[:, col : col + 1], scale=1.0,
                    )
                else:
                    nc.gpsimd.tensor_scalar(
                        out=tile_, in0=sb_ct[:, 0 : tile_.shape[1]],
                        scalar1=sb_ap[:, col : col + 1], scalar2=0.0,
                        op0=add, op1=mx,
                    )

            # prologue quad 0 on DVE, sized to DMA piece arrival: 512-col
            # ops first so chunk-0 matmuls start as soon as cT piece 0 lands
            a0, b0 = h1(0)
            for lo, hi in ((0, 512), (512, 1024), (1024, 2048), (2048, 4096)):
                for tile_, col in ((a0, 0), (b0, 1)):
                    nc.vector.tensor_scalar(
                        out=tile_[:, lo:hi], in0=sb_ct[:, lo:hi],
                        scalar1=sb_ap[:, col : col + 1], scalar2=0.0,
                        op0=add, op1=mx,
                    )

            prev_h2s = None  # previous quad's h2 tiles, G2 lagged a quad
            for q in range(NQUAD):
                h1a, h1b = h1(q)
                # DVE's pass2 chunk share: 2.5 avg in steady state (balanced
                # against its pass1 load), more in the last quads once pass1
                # emission dries up and ACT would otherwise become the tail.
                if q == NQUAD - 1:
                    dve_set = (1, 3, 5, 7)
                elif q == NQUAD - 2:
                    dve_set = (2, 5, 7)
                else:
                    # 2.5 DVE chunks/quad, strictly alternating. The LP
                    # balance point is 2.67, but patterns with consecutive
                    # 3-chunk DVE quads measured 5%% slower: DVE pass2
                    # backlog delays the 3-slot PSUM ring recycle and stalls
                    # the PE (and then ACT) behind it.
                    dve_set = (2, 5) if q % 2 == 0 else (2, 5, 7)
                h2s = []
                for c in range(NCH):
                    sl = slice(c * CHW, (c + 1) * CHW)
                    ps = pspool.tile([128, 2 * CHW], f32, tag="h2pre")
                    nc.tensor.matmul(
                        ps[0:64, 0:CHW], sb_g1w[0:64, :], h1a[0:64, sl],
                        start=True, stop=True, tile_position=(0, 0),
                    )
                    nc.tensor.matmul(
                        ps[64:128, 0:CHW], sb_g1w[64:128, :], h1a[64:128, sl],
                        start=True, stop=True, tile_position=(64, 64),
                    )
                    nc.tensor.matmul(
                        ps[64:128, CHW : 2 * CHW], sb_g1w[0:64, :], h1b[0:64, sl],
                        start=True, stop=True, tile_position=(0, 64),
                    )
                    nc.tensor.matmul(
                        ps[0:64, CHW : 2 * CHW], sb_g1w[64:128, :], h1b[64:128, sl],
                        start=True, stop=True, tile_position=(64, 0),
                    )
                    h2 = h2pool.tile([128, 2 * CHW], f16, tag="h2")
                    if c in dve_set:
                        nc.vector.tensor_scalar(
                            out=h2[:], in0=ps[:],
                            scalar1=sb_g1b, scalar2=0.0, op0=add, op1=mx,
                        )
                    else:
                        nc.scalar.activation(
                            out=h2[:], in_=ps[:], func=Relu,
                            bias=sb_g1b, scale=1.0,
                        )
                    h2s.append(h2)
                    if c == 1 and q + 1 < NQUAD:
                        pass1("V", h1(q + 1)[0][:], 2 * (q + 1))
                    if c == 4 and q + 1 < NQUAD:
                        pass1("V", h1(q + 1)[1][:], 2 * (q + 1) + 1)
                    if c == 1 and prev_h2s is not None:
                        emit_g2_batch(q - 1, 4, prev_h2s[4:8])
                emit_g2_batch(q, 0, h2s[0:4])
                prev_h2s = h2s

            def evac_u(i):
                so = outpool.tile([128, CHW], f32, name=f"so{i}", tag=f"so{i}")
                if i == 0:
                    nc.vector.tensor_copy(out=so[:], in_=psu[i][:])
                else:
                    nc.scalar.copy(out=so[:], in_=psu[i][:])
                nc.sync.dma_start(
                    out=d_u[4 * i : 4 * i + 4, :], in_=so[::32, :]
                )

            # final U1 accumulation in two 4-MM column-group batches so the
            # (c4,c5) pair starts as soon as its pass2 lands instead of
            # waiting for c7's; U0 evac overlaps both.
            evac_u(0)
            for half in range(2):
                for k in (0, 1):
                    nc.tensor.matmul(
                        psu[1][32 * k : 32 * k + 1, :], sb_g2w,
                        prev_h2s[4 + k][:, half * CHW : (half + 1) * CHW],
                        start=False, stop=(half == 1),
                        tile_position=(0, 32 * k),
                    )
            for half in range(2):
                for k in (2, 3):
                    nc.tensor.matmul(
                        psu[1][32 * k : 32 * k + 1, :], sb_g2w,
                        prev_h2s[4 + k][:, half * CHW : (half + 1) * CHW],
                        start=False, stop=(half == 1),
                        tile_position=(0, 32 * k),
                    )
            evac_u(1)

    nc.compile()
    return nc


def _relu(x):
    return np.maximum(x, 0.0)


def _prepare_in_maps(
    boundary_info, interior_coords, W0, b0, W1, b1,
    G0w, G0b, G1w, G1b, G2w, G2b,
):
    f16 = np.float16
    bi = np.asarray(boundary_info, np.float32)
    coords = np.asarray(interior_coords, np.float32)
    W0, b0, W1, b1 = (np.asarray(t, np.float32) for t in (W0, b0, W1, b1))
    G0w, G0b, G1w, G1b, G2w, G2b = (
        np.asarray(t, np.float32) for t in (G0w, G0b, G1w, G1b, G2w, G2b)
    )

    # tiny encoder stages on host
    bf = _relu(bi @ W0 + b0)
    bf = _relu(bf @ W1 + b1)
    a = bf @ G0w[:HID] + G0b  # (B, NBC, HID)
    cint = coords @ G0w[HID:]  # (B, NINT, HID)

    g1w_sb = np.vstack([G1w, G1w]).astype(f16)
    g2w_sb = np.vstack([G2w, G2w]).astype(f16)
    g1b2 = np.concatenate([G1b, G1b]).reshape(128, 1).astype(np.float32)
    csth = np.ascontiguousarray(np.concatenate([g1w_sb, g2w_sb], axis=1))

    in_maps = []
    for core in range(NCORES):
        b, half = divmod(core, 2)
        cT = np.ascontiguousarray(cint[b].T)  # (64, 4096)
        ctdup = np.vstack([cT, cT]).astype(f16)
        asl = a[b, half * 64 : (half + 1) * 64]  # (64 bc, 64 hid)
        apairs = np.ascontiguousarray(asl.reshape(32, 128).T).astype(np.float32)
        cstf = np.ascontiguousarray(np.concatenate([apairs, g1b2], axis=1))
        in_maps.append({"ctdup": ctdup, "cstf": cstf, "csth": csth})
    return in_maps


def _run(in_maps, **kwargs):
    from concourse.bass_utils import run_bass_kernel_spmd

    if "nc" not in _PROG:
        _PROG["nc"] = _build_program()
    return run_bass_kernel_spmd(_PROG["nc"], in_maps, list(range(NCORES)), **kwargs)


def kernel(
    boundary_info, interior_coords, W0, b0, W1, b1,
    G0w, G0b, G1w, G1b, G2w, G2b, interior_h, interior_w,
):
    in_maps = _prepare_in_maps(
        boundary_info, interior_coords, W0, b0, W1, b1,
        G0w, G0b, G1w, G1b, G2w, G2b,
    )
    res = _run(in_maps)

    u = np.zeros((B, NINT), np.float64)
    for core in range(NCORES):
        b = core // 2
        u[b] += res.results[core]["upart"].reshape(NINT).astype(np.float64)
    u = (u / NBC + np.asarray(G2b, np.float32)[0]).astype(np.float32)
    return u.reshape(B, 1, int(interior_h), int(interior_w))


# revision 27
# speedup vs baseline: 1.0634x; 1.0184x over previous
"""Trainium2 Bass kernel for nn_BoundaryGreenBranch.

Math (reference):
    bf = relu(relu(bi @ W0 + b0) @ W1 + b1)            # (B, NBC, HID) tiny
    a  = bf @ G0w[:HID] + G0b                          # (B, NBC, HID) tiny
    c  = coords @ G0w[HID:]                            # (B, NINT, HID) small
    h1 = relu(a[:,:,None,:] + c[:,None,:,:])           # (B, NBC, NINT, HID) huge
    h2 = relu(h1 @ G1w + G1b)                          # huge
    u  = (h2 @ G2w + G2b).sum(bc) / NBC                # (B, NINT, 1)

Sharding: 8 cores = 4 batches x 2 halves of NBC (64 bc each). Host does the
tiny encoder stages; each core does its 64bc x 4096int x 64hid block fully
on-chip; host sums the two partial u's per batch (the bc all-reduce).

On-core layout (per quad of 4 bc, pairs packed 2-up on 128 partitions):
    pass1: h1 tiles [128, 4096] fp16, relu(cT_dup + a'_pair) with a' as a
        per-partition scalar: DVE tensor_scalar (4x mode) for most tiles,
        ACT for the prologue h1b (fills ACT's idle startup window), GPSIMD
        for a few h1b tiles (emitted 2 quads ahead to cover its latency).
    G1: 4 concurrent quadrant matmuls (tile_position) since K=M=64 fills the
        128x128 PE array -> h2pre in PSUM [128, 1024] (2 banks, 3 slots).
    pass2: relu(h2pre + G1b) PSUM->SBUF fp16, split chunk-by-chunk between
        ACT (activation bias trick, ~1147ns) and DVE (dual-op tensor_scalar,
        ~1270ns) by a greedy virtual-clock balance that also accounts for
        pass1 -- so DVE picks up extra chunks in the last quads when pass1
        runs dry instead of idling.
    G2: lhsT=[G2w;G2w] [128,1] matmuls accumulate the sum over bc in PSUM u
        slots (8 chunks -> 2 banks x 4 col-group partitions); emitted in
        4-column-group concurrent batches, lagged a quad so the PE never
        waits on a late pass2. fp16 everywhere on the 16-bit path: same
        speed as bf16 but ~8x lower error (~4e-4 rel).

Small constants ride in two DMAs (f32 a-pairs+G1b, f16 G1w+G2w); cT goes as
8 x 512-col pieces split over the sync+gpsimd issue queues. Output u leaves
as one strided-partition DMA per PSUM accumulator bank.
"""

import numpy as np

B, NBC, HID = 4, 128, 64
NINT = 4096
NCORES = 8
NQUAD = 16  # quads of 4 bc per core (64 bc / 4)
NCH = 8  # interior chunks of 512
CHW = 512  # chunk width

# GPSIMD's stock tensor_scalar ucode measures ~17 cyc/elem (58us per h1
# tile) -- useless for pass1, so no quads are assigned to it.
GP_QUADS = frozenset()

_PROG = {}


def _build_program():
    import concourse.bacc as bacc
    import concourse.tile as tile
    from concourse import mybir

    f32 = mybir.dt.float32
    f16 = mybir.dt.float16
    Relu = mybir.ActivationFunctionType.Relu
    add = mybir.AluOpType.add
    mx = mybir.AluOpType.max

    nc = bacc.Bacc("TRN2")
    d_ct = nc.declare_dram_parameter("ctdup", [128, NINT], f16, isOutput=False)
    d_cstf = nc.declare_dram_parameter("cstf", [128, 33], f32, isOutput=False)
    d_csth = nc.declare_dram_parameter("csth", [128, 65], f16, isOutput=False)
    d_u = nc.declare_dram_parameter("upart", [NCH, CHW], f32, isOutput=True)

    with tile.TileContext(nc) as tc:
        with (
            tc.tile_pool(name="const", bufs=1) as const,
            tc.tile_pool(name="h1", bufs=3) as h1pool,
            tc.tile_pool(name="h2", bufs=14) as h2pool,
            tc.tile_pool(name="ps", bufs=3, space="PSUM") as pspool,
            tc.tile_pool(name="psu", bufs=1, space="PSUM") as upool,
            tc.tile_pool(name="outp", bufs=1) as outpool,
        ):
            # DMA order: the first 512 cT columns gate the whole pipeline,
            # so they go as two 256-col DMAs in parallel on sync+gpsimd;
            # the small consts ride the otherwise-idle scalar queue.
            cstf = const.tile([128, 33], f32)
            csth = const.tile([128, 65], f16)
            sb_ct = const.tile([128, NINT], f16)

            def ctp(i):
                return slice(i * 512, (i + 1) * 512)

            nc.sync.dma_start(out=sb_ct[:, ctp(0)], in_=d_ct[:, ctp(0)])
            nc.scalar.dma_start(out=cstf[:], in_=d_cstf[:])
            nc.scalar.dma_start(out=csth[:], in_=d_csth[:])
            for i in (2, 4, 6):
                nc.sync.dma_start(out=sb_ct[:, ctp(i)], in_=d_ct[:, ctp(i)])
            for i in (1, 3, 5, 7):
                nc.gpsimd.dma_start(out=sb_ct[:, ctp(i)], in_=d_ct[:, ctp(i)])

            sb_ap = cstf[:, 0:32]
            sb_g1b = cstf[:, 32:33]
            sb_g1w = csth[:, 0:64]
            sb_g2w = csth[:, 64:65]

            # warm the ACT Relu table while the cT DMA runs
            dummy = const.tile([128, 1], f32)
            nc.scalar.activation(out=dummy[:], in_=sb_g1b, func=Relu)

            psu = [
                upool.tile([128, CHW], f32, name=f"u{i}", tag=f"u{i}")
                for i in range(2)
            ]

            def emit_g2_batch(q, cbase, h2s4):
                """8 G2 matmuls for chunks cbase..cbase+3: per tile-half, the
                4 chunks target 4 distinct PE column groups -> concurrent.
                Keep these batched: G2's full-height column LDWEIGHTS
                conflicts with the G1 quadrants, so each batch is one PE
                serialization point (spreading them per-chunk measured 10%
                slower end-to-end)."""
                ub = psu[cbase // 4]
                for half in range(2):
                    sl = slice(half * CHW, (half + 1) * CHW)
                    for k in range(4):
                        j = 32 * k
                        nc.tensor.matmul(
                            ub[j : j + 1, :], sb_g2w, h2s4[k][:, sl],
                            start=(q == 0 and half == 0),
                            stop=(q == NQUAD - 1 and half == 1),
                            tile_position=(0, j),
                        )

            h1t = {}

            def h1(qq):
                if qq not in h1t:
                    h1t[qq] = (
                        h1pool.tile([128, NINT], f16, name="h1a", tag="h1a"),
                        h1pool.tile([128, NINT], f16, name="h1b", tag="h1b"),
                    )
                return h1t[qq]

            def pass1(eng, tile_, col):
                if eng == "V":
                    nc.vector.tensor_scalar(
                        out=tile_, in0=sb_ct[:, 0 : tile_.shape[1]],
                        scalar1=sb_ap[:, col : col + 1], scalar2=0.0,
                        op0=add, op1=mx,
                    )
                elif eng == "A":
                    nc.scalar.activation(
                        out=tile_, in_=sb_ct[:, 0 : tile_.shape[1]], func=Relu,
                        bias=sb_ap[:, col : col + 1], scale=1.0,
                    )
                else:
                    nc.gpsimd.tensor_scalar(
                        out=tile_, in0=sb_ct[:, 0 : tile_.shape[1]],
                        scalar1=sb_ap[:, col : col + 1], scalar2=0.0,
                        op0=add, op1=mx,
                    )

            # prologue quad 0 on DVE, sized to DMA piece arrival: 512-col
            # ops first so chunk-0 matmuls start as soon as cT piece 0 lands
            a0, b0 = h1(0)
            for lo, hi in ((0, 512), (512, 1024), (1024, 2048), (2048, 4096)):
                for tile_, col in ((a0, 0), (b0, 1)):
                    nc.vector.tensor_scalar(
                        out=tile_[:, lo:hi], in0=sb_ct[:, lo:hi],
                        scalar1=sb_ap[:, col : col + 1], scalar2=0.0,
                        op0=add, op1=mx,
                    )

            prev_h2s = None  # previous quad's h2 tiles, G2 lagged a quad
            for q in range(NQUAD):
                h1a, h1b = h1(q)
                # DVE's pass2 chunk share: 2.5 avg in steady state (balanced
                # against its pass1 load), more in the last quads once pass1
                # emission dries up and ACT would otherwise become the tail.
                if q == NQUAD - 1:
                    dve_set = (1, 3, 5, 7)
                elif q == NQUAD - 2:
                    dve_set = (2, 5, 7)
                else:
                    # 2.5 DVE chunks/quad, strictly alternating. The LP
                    # balance point is 2.67, but patterns with consecutive
                    # 3-chunk DVE quads measured 5%% slower: DVE pass2
                    # backlog delays the 3-slot PSUM ring recycle and stalls
                    # the PE (and then ACT) behind it.
                    dve_set = (2, 5) if q % 2 == 0 else (2, 5, 7)
                h2s = []
                for c in range(NCH):
                    sl = slice(c * CHW, (c + 1) * CHW)
                    ps = pspool.tile([128, 2 * CHW], f32, tag="h2pre")
                    nc.tensor.matmul(
                        ps[0:64, 0:CHW], sb_g1w[0:64, :], h1a[0:64, sl],
                        start=True, stop=True, tile_position=(0, 0),
                    )
                    nc.tensor.matmul(
                        ps[64:128, 0:CHW], sb_g1w[64:128, :], h1a[64:128, sl],
                        start=True, stop=True, tile_position=(64, 64),
                    )
                    nc.tensor.matmul(
                        ps[64:128, CHW : 2 * CHW], sb_g1w[0:64, :], h1b[0:64, sl],
                        start=True, stop=True, tile_position=(0, 64),
                    )
                    nc.tensor.matmul(
                        ps[0:64, CHW : 2 * CHW], sb_g1w[64:128, :], h1b[64:128, sl],
                        start=True, stop=True, tile_position=(64, 0),
                    )
                    h2 = h2pool.tile([128, 2 * CHW], f16, tag="h2")
                    if c in dve_set:
                        nc.vector.tensor_scalar(
                            out=h2[:], in0=ps[:],
                            scalar1=sb_g1b, scalar2=0.0, op0=add, op1=mx,
                        )
                    else:
                        nc.scalar.activation(
                            out=h2[:], in_=ps[:], func=Relu,
                            bias=sb_g1b, scale=1.0,
                        )
                    h2s.append(h2)
                    if c == 1 and q + 1 < NQUAD:
                        pass1("V", h1(q + 1)[0][:], 2 * (q + 1))
                    if c == 4 and q + 1 < NQUAD:
                        pass1("V", h1(q + 1)[1][:], 2 * (q + 1) + 1)
                    if c == 1 and prev_h2s is not None:
                        emit_g2_batch(q - 1, 4, prev_h2s[4:8])
                emit_g2_batch(q, 0, h2s[0:4])
                prev_h2s = h2s

            def evac_u(i):
                so = outpool.tile([128, CHW], f32, name=f"so{i}", tag=f"so{i}")
                if i == 0:
                    nc.vector.tensor_copy(out=so[:], in_=psu[i][:])
                else:
                    nc.scalar.copy(out=so[:], in_=psu[i][:])
                nc.sync.dma_start(
                    out=d_u[4 * i : 4 * i + 4, :], in_=so[::32, :]
                )

            # final U1 accumulation in two 4-MM column-group batches so the
            # (c4,c5) pair starts as soon as its pass2 lands instead of
            # waiting for c7's; U0 evac overlaps both.
            evac_u(0)
            for half in range(2):
                for k in (0, 1):
                    nc.tensor.matmul(
                        psu[1][32 * k : 32 * k + 1, :], sb_g2w,
                        prev_h2s[4 + k][:, half * CHW : (half + 1) * CHW],
                        start=False, stop=(half == 1),
                        tile_position=(0, 32 * k),
                    )
            for half in range(2):
                for k in (2, 3):
                    nc.tensor.matmul(
                        psu[1][32 * k : 32 * k + 1, :], sb_g2w,
                        prev_h2s[4 + k][:, half * CHW : (half + 1) * CHW],
                        start=False, stop=(half == 1),
                        tile_position=(0, 32 * k),
                    )
            evac_u(1)

    nc.compile()
    return nc


def _relu(x):
    return np.maximum(x, 0.0)


def _prepare_in_maps(
    boundary_info, interior_coords, W0, b0, W1, b1,
    G0w, G0b, G1w, G1b, G2w, G2b,
):
    f16 = np.float16
    bi = np.asarray(boundary_info, np.float32)
    coords = np.asarray(interior_coords, np.float32)
    W0, b0, W1, b1 = (np.asarray(t, np.float32) for t in (W0, b0, W1, b1))
    G0w, G0b, G1w, G1b, G2w, G2b = (
        np.asarray(t, np.float32) for t in (G0w, G0b, G1w, G1b, G2w, G2b)
    )

    # tiny encoder stages on host
    bf = _relu(bi @ W0 + b0)
    bf = _relu(bf @ W1 + b1)
    a = bf @ G0w[:HID] + G0b  # (B, NBC, HID)
    cint = coords @ G0w[HID:]  # (B, NINT, HID)

    g1w_sb = np.vstack([G1w, G1w]).astype(f16)
    g2w_sb = np.vstack([G2w, G2w]).astype(f16)
    g1b2 = np.concatenate([G1b, G1b]).reshape(128, 1).astype(np.float32)
    csth = np.ascontiguousarray(np.concatenate([g1w_sb, g2w_sb], axis=1))

    in_maps = []
    for core in range(NCORES):
        b, half = divmod(core, 2)
        cT = np.ascontiguousarray(cint[b].T)  # (64, 4096)
        ctdup = np.vstack([cT, cT]).astype(f16)
        asl = a[b, half * 64 : (half + 1) * 64]  # (64 bc, 64 hid)
        apairs = np.ascontiguousarray(asl.reshape(32, 128).T).astype(np.float32)
        cstf = np.ascontiguousarray(np.concatenate([apairs, g1b2], axis=1))
        in_maps.append({"ctdup": ctdup, "cstf": cstf, "csth": csth})
    return in_maps


def _run(in_maps, **kwargs):
    from concourse.bass_utils import run_bass_kernel_spmd

    if "nc" not in _PROG:
        _PROG["nc"] = _build_program()
    return run_bass_kernel_spmd(_PROG["nc"], in_maps, list(range(NCORES)), **kwargs)


def kernel(
    boundary_info, interior_coords, W0, b0, W1, b1,
    G0w, G0b, G1w, G1b, G2w, G2b, interior_h, interior_w,
):
    in_maps = _prepare_in_maps(
        boundary_info, interior_coords, W0, b0, W1, b1,
        G0w, G0b, G1w, G1b, G2w, G2b,
    )
    res = _run(in_maps)

    u = np.zeros((B, NINT), np.float64)
    for core in range(NCORES):
        b = core // 2
        u[b] += res.results[core]["upart"].reshape(NINT).astype(np.float64)
    u = (u / NBC + np.asarray(G2b, np.float32)[0]).astype(np.float32)
    return u.reshape(B, 1, int(interior_h), int(interior_w))
